# revision 10
# baseline (speedup 1.0000x reference)
"""Bass/Trainium2 kernel for nn_HALTON_33277406609678 (ragged_sequence).

Reference computation:
    feat[b] = max over compacted-valid positions p in [s_b, e_b] of
              (p-th valid token of enc[b] if p < num_valid_b else 0)
    out = relu(feat @ W1 + b1) @ W2 + b2

pos_span values live in [0, 40), so at most the first 48 (padded) valid
tokens of a row matter.  The host (cheap: indexing + dtype conversion
only) gathers those token rows per batch row into a dense fp16 tensor
laid out TRANSPOSED per D-chunk, so the device's span-max is a plain
strided reduce_max straight into the matmul's stationary layout -- no
indirect DMA, no PE transposes of gathered data.

Sharding: pure data parallel -- 8 batch rows per core, head weights
replicated (fp16).  b2 is added on the host (64x128 adds).

Slot semantics (host): slot j of row b holds compacted position q=s+j:
  real token       if q <= e and q <  nv
  zero row         if q <= e and q >= nv   (reference pools zeros there)
  dup of slot 0    if q >  e                (padding; never raises max)
If s >= nv the whole span is zero rows -> feat = 0 and the device MLP
yields relu(b1) @ W2 organically; no host patching needed.
"""

import numpy as np

B, L, D, H, K = 64, 512, 768, 768, 128
NCORES = 8
RPC = B // NCORES          # rows per core
CH = D // 128              # 128-wide chunks of D / H (= 6)

_CACHE = {}


def _build_nc(SLOTS):
    import concourse.bass as bass
    import concourse.bacc as bacc
    import concourse.mybir as mybir
    import concourse.tile as tile
    from concourse.masks import make_identity
    from concourse.tile_rust import add_dep_helper
    from contextlib import ExitStack

    f16 = mybir.dt.float16
    f32 = mybir.dt.float32

    nc = bacc.Bacc(
        "TRN2", target_bir_lowering=False, debug=False, num_devices=NCORES
    )
    GW = RPC * SLOTS  # gather cols per D-chunk
    g_d = nc.dram_tensor("g", [128, CH * GW], f16, kind="ExternalInput")
    b1_d = nc.dram_tensor("b1", [128, CH], f32, kind="ExternalInput")
    w1_d = nc.dram_tensor("w1", [128, CH * H], f16, kind="ExternalInput")
    w2_d = nc.dram_tensor("w2", [128, CH * K], f16, kind="ExternalInput")
    out_d = nc.dram_tensor("out", [RPC, K], f16, kind="ExternalOutput")

    with tile.TileContext(nc) as tc, ExitStack() as ctx:
        cpool = ctx.enter_context(tc.tile_pool(name="const", bufs=1))
        spool = ctx.enter_context(tc.tile_pool(name="scratch", bufs=2))
        ppool_h = ctx.enter_context(tc.tile_pool(name="ph", bufs=1, space="PSUM"))
        ppool_t = ctx.enter_context(tc.tile_pool(name="pt", bufs=2, space="PSUM"))
        ppool_l = ctx.enter_context(tc.tile_pool(name="pl", bufs=1, space="PSUM"))

        # sync HWDGE ring: the gathered tokens first -- they gate everything.
        g_sb = cpool.tile([128, CH * GW], f16, tag="g")
        g_inst = nc.sync.dma_start(g_sb[:], g_d[:])
        b1_sb = cpool.tile([128, CH], f32, tag="b1")
        nc.sync.dma_start(b1_sb[:], b1_d[:])

        # scalar (ACT) HWDGE ring: W1 in 3 pipelined parts, W2 behind them.
        # W1 waits for G so the gather gets the full HBM pipe first.
        w1_sb = cpool.tile([128, CH * H], f16, tag="w1")
        NPART = 3
        CPP = CH // NPART  # chunks per part
        for part in range(NPART):
            cols = slice(part * CPP * H, (part + 1) * CPP * H)
            w1_inst = nc.scalar.dma_start(w1_sb[:, cols], w1_d[:, cols])
            if part == 0:
                add_dep_helper(w1_inst.ins, g_inst.ins, sync=True,
                               reason="gather gets the HBM pipe first")
        w2_sb = cpool.tile([128, CH * K], f16, tag="w2")
        nc.scalar.dma_start(w2_sb[:], w2_d[:])

        ident = cpool.tile([128, 128], f32, tag="ident")
        make_identity(nc, ident[:])

        # feat_c[d, r] = max over slots j of g[d, (c r j)]
        feat = []
        for c in range(CH):
            f = cpool.tile([128, RPC], f16, tag=f"feat{c}")
            nc.vector.reduce_max(
                f[:],
                g_sb[:, c * GW:(c + 1) * GW].rearrange("p (r j) -> p r j", j=SLOTS),
                axis=mybir.AxisListType.X,
            )
            feat.append(f)

        # h = feat @ W1 : [RPC, H] in two 384-wide PSUM halves; chunk-major
        # order so each W1 part unlocks its matmuls as it lands.
        NH = H // 2
        h_ps0 = ppool_h.tile([RPC, NH], f32, tag="h0")
        h_ps1 = ppool_h.tile([RPC, NH], f32, tag="h1")
        h_ps = [h_ps0, h_ps1]
        for kc in range(CH):
            for half in range(2):
                nc.tensor.matmul(
                    out=h_ps[half][:],
                    lhsT=feat[kc][:],
                    rhs=w1_sb[:, kc * H + half * NH: kc * H + (half + 1) * NH],
                    start=(kc == 0),
                    stop=(kc == CH - 1),
                )
        h_sb = spool.tile([RPC, H], f32, tag="hsb")
        for half in range(2):
            nc.scalar.copy(h_sb[:, half * NH:(half + 1) * NH], h_ps[half][:])

        # per H-chunk: transpose -> relu(x + b1) -> logits matmul accumulate
        l_ps = ppool_l.tile([RPC, K], f32, tag="l")
        for hc in range(CH):
            ht_ps = ppool_t.tile([128, RPC], f32, tag="htp")
            nc.tensor.transpose(
                out=ht_ps[:], in_=h_sb[:, hc * 128:(hc + 1) * 128],
                identity=ident[:RPC, :RPC],
            )
            ht = spool.tile([128, RPC], f16, tag=f"ht{hc}")
            nc.scalar.activation(
                ht[:], ht_ps[:], mybir.ActivationFunctionType.Relu,
                bias=b1_sb[:, hc:hc + 1],
            )
            nc.tensor.matmul(
                out=l_ps[:],
                lhsT=ht[:],
                rhs=w2_sb[:, hc * K:(hc + 1) * K],
                start=(hc == 0),
                stop=(hc == CH - 1),
            )
        out_sb = spool.tile([RPC, K], f16, tag="out")
        nc.vector.tensor_copy(out_sb[:], l_ps[:])
        nc.sync.dma_start(out_d[:], out_sb[:])

    nc.compile()
    return nc


def _get_nc(slots):
    if slots not in _CACHE:
        _CACHE[slots] = _build_nc(slots)
    return _CACHE[slots]


def _pick_slots(valid_mask, pos_span):
    """Slot count for this input: longest span that can touch real/zero rows."""
    span = np.asarray(pos_span).astype(np.int64)
    ln = int((span[:, 1] - span[:, 0]).max()) + 1
    return max(8, (ln + 3) // 4 * 4)


def _host_gather(enc16, valid_mask, pos_span, SLOTS):
    """Dense [B, SLOTS] token values per the slot semantics above -> fp16."""
    v = np.asarray(valid_mask).astype(np.int64) == 1          # [B, L]
    span = np.asarray(pos_span).astype(np.int64)              # [B, 2]
    s, e = span[:, 0], span[:, 1]
    nv = v.sum(axis=1)                                        # num valid per row
    order = np.argsort(~v, axis=1, kind="stable")             # valid tokens first
    q = s[:, None] + np.arange(SLOTS)[None, :]                # rank per slot
    qc = np.where(q <= e[:, None], q, s[:, None])             # padding -> slot 0
    use_zero = qc >= nv[:, None]                              # [B, SLOTS]
    toks = np.take_along_axis(order, np.minimum(qc, L - 1), axis=1)
    vals = enc16[np.arange(B)[:, None], toks]                 # [B, SLOTS, D]
    vals[use_zero] = np.float16(0.0)
    return vals


def _make_in_maps(inputs):
    enc16 = np.asarray(inputs["encoder_layers"], dtype=np.float32).astype(np.float16)
    W1 = np.asarray(inputs["W1"], dtype=np.float32)
    b1 = np.asarray(inputs["b1"], dtype=np.float32)
    W2 = np.asarray(inputs["W2"], dtype=np.float32)

    SLOTS = _pick_slots(inputs["valid_mask"], inputs["pos_span"])
    vals = _host_gather(enc16, inputs["valid_mask"], inputs["pos_span"], SLOTS)

    # device layouts: partition = d % 128, free = (chunk, ...)
    w1_dev = np.ascontiguousarray(
        W1.astype(np.float16).reshape(CH, 128, H).transpose(1, 0, 2).reshape(128, CH * H))
    w2_dev = np.ascontiguousarray(
        W2.astype(np.float16).reshape(CH, 128, K).transpose(1, 0, 2).reshape(128, CH * K))
    b1_dev = np.ascontiguousarray(b1.reshape(CH, 128).T)      # [128, CH] f32

    in_maps = []
    for c in range(NCORES):
        rows = slice(c * RPC, (c + 1) * RPC)
        # g[d%128, (c r j)] = vals[r, j, d]
        g = (vals[rows]                                       # [RPC, SLOTS, D]
             .transpose(2, 0, 1)                              # [D, RPC, SLOTS]
             .reshape(CH, 128, RPC * SLOTS)
             .transpose(1, 0, 2)
             .reshape(128, CH * RPC * SLOTS))
        in_maps.append({
            "g": np.ascontiguousarray(g),
            "b1": b1_dev, "w1": w1_dev, "w2": w2_dev,
        })
    return in_maps, SLOTS


def kernel(**inputs):
    from concourse.bass_utils import run_bass_kernel_spmd

    in_maps, slots = _make_in_maps(inputs)
    nc = _get_nc(slots)
    res = run_bass_kernel_spmd(nc, in_maps, list(range(NCORES)))
    out = np.concatenate(
        [res.results[c]["out"].astype(np.float32) for c in range(NCORES)], axis=0)

    b2 = np.asarray(inputs["b2"], dtype=np.float32)
    return (out + b2[None, :]).astype(np.float32)


# revision 14
# speedup vs baseline: 1.1008x; 1.1008x over previous
"""Bass/Trainium2 kernel for nn_HALTON_33277406609678 (ragged_sequence).

Reference computation:
    feat[b] = max over compacted-valid positions p in [s_b, e_b] of
              (p-th valid token of enc[b] if p < num_valid_b else 0)
    out = relu(feat @ W1 + b1) @ W2 + b2

pos_span values live in [0, 40), so at most the first 48 (padded) valid
tokens of a row matter.  The host (cheap: indexing + dtype conversion
only) gathers those token rows per batch row into a dense fp16 tensor
laid out TRANSPOSED per D-chunk, so the device's span-max is a plain
strided reduce_max straight into the matmul's stationary layout -- no
indirect DMA, no PE transposes of gathered data.

Sharding: pure data parallel -- 8 batch rows per core, head weights
replicated (fp16).  b2 is added on the host (64x128 adds).

Slot semantics (host): slot j of row b holds compacted position q=s+j:
  real token       if q <= e and q <  nv
  zero row         if q <= e and q >= nv   (reference pools zeros there)
  dup of slot 0    if q >  e                (padding; never raises max)
If s >= nv the whole span is zero rows -> feat = 0 and the device MLP
yields relu(b1) @ W2 organically; no host patching needed.
"""

import numpy as np

B, L, D, H, K = 64, 512, 768, 768, 128
NCORES = 8
RPC = B // NCORES          # rows per core
CH = D // 128              # 128-wide chunks of D / H (= 6)

_CACHE = {}


def _build_nc(SLOTS):
    import concourse.bass as bass
    import concourse.bacc as bacc
    import concourse.mybir as mybir
    import concourse.tile as tile
    from concourse.masks import make_identity
    from contextlib import ExitStack

    f16 = mybir.dt.float16
    f32 = mybir.dt.float32

    nc = bacc.Bacc(
        "TRN2", target_bir_lowering=False, debug=False, num_devices=NCORES
    )
    GW = RPC * SLOTS  # gather cols per D-chunk
    g_d = nc.dram_tensor("g", [128, CH * GW], f16, kind="ExternalInput")
    b1_d = nc.dram_tensor("b1", [128, CH], f32, kind="ExternalInput")
    w1_d = nc.dram_tensor("w1", [128, CH * H], f16, kind="ExternalInput")
    w2_d = nc.dram_tensor("w2", [128, CH * K], f16, kind="ExternalInput")
    out_d = nc.dram_tensor("out", [RPC, K], f16, kind="ExternalOutput")

    with tile.TileContext(nc) as tc, ExitStack() as ctx:
        cpool = ctx.enter_context(tc.tile_pool(name="const", bufs=1))
        spool = ctx.enter_context(tc.tile_pool(name="scratch", bufs=2))
        ppool_h = ctx.enter_context(tc.tile_pool(name="ph", bufs=1, space="PSUM"))
        ppool_t = ctx.enter_context(tc.tile_pool(name="pt", bufs=2, space="PSUM"))
        ppool_l = ctx.enter_context(tc.tile_pool(name="pl", bufs=1, space="PSUM"))

        # sync HWDGE ring: the gathered tokens first -- they gate everything.
        g_sb = cpool.tile([128, CH * GW], f16, tag="g")
        nc.sync.dma_start(g_sb[:], g_d[:])
        b1_sb = cpool.tile([128, CH], f32, tag="b1")
        nc.sync.dma_start(b1_sb[:], b1_d[:])

        # scalar (ACT) HWDGE ring: W1 in 3 pipelined parts, W2 behind them.
        # W1 waits for G so the gather gets the full HBM pipe first.
        w1_sb = cpool.tile([128, CH * H], f16, tag="w1")
        NPART = 3
        CPP = CH // NPART  # chunks per part
        for part in range(NPART):
            cols = slice(part * CPP * H, (part + 1) * CPP * H)
            nc.scalar.dma_start(w1_sb[:, cols], w1_d[:, cols])
        w2_sb = cpool.tile([128, CH * K], f16, tag="w2")
        nc.scalar.dma_start(w2_sb[:], w2_d[:])

        ident = cpool.tile([128, 128], f32, tag="ident")
        make_identity(nc, ident[:])

        # feat_c[d, r] = max over slots j of g[d, (c r j)]
        feat = []
        for c in range(CH):
            f = cpool.tile([128, RPC], f16, tag=f"feat{c}")
            nc.vector.reduce_max(
                f[:],
                g_sb[:, c * GW:(c + 1) * GW].rearrange("p (r j) -> p r j", j=SLOTS),
                axis=mybir.AxisListType.X,
            )
            feat.append(f)

        # h = feat @ W1 : [RPC, H] in two 384-wide PSUM halves; chunk-major
        # order so each W1 part unlocks its matmuls as it lands.
        NH = H // 2
        h_ps0 = ppool_h.tile([RPC, NH], f32, tag="h0")
        h_ps1 = ppool_h.tile([RPC, NH], f32, tag="h1")
        h_ps = [h_ps0, h_ps1]
        for kc in range(CH):
            for half in range(2):
                nc.tensor.matmul(
                    out=h_ps[half][:],
                    lhsT=feat[kc][:],
                    rhs=w1_sb[:, kc * H + half * NH: kc * H + (half + 1) * NH],
                    start=(kc == 0),
                    stop=(kc == CH - 1),
                )
        h_sb = spool.tile([RPC, H], f32, tag="hsb")
        for half in range(2):
            nc.vector.tensor_copy(h_sb[:, half * NH:(half + 1) * NH], h_ps[half][:])

        # per H-chunk: transpose -> relu(x + b1) -> logits matmul accumulate
        l_ps = ppool_l.tile([RPC, K], f32, tag="l")
        for hc in range(CH):
            ht_ps = ppool_t.tile([128, RPC], f32, tag="htp")
            nc.tensor.transpose(
                out=ht_ps[:], in_=h_sb[:, hc * 128:(hc + 1) * 128],
                identity=ident[:RPC, :RPC],
            )
            ht = spool.tile([128, RPC], f16, tag=f"ht{hc}")
            nc.vector.tensor_scalar(
                out=ht[:], in0=ht_ps[:], scalar1=b1_sb[:, hc:hc + 1], scalar2=0.0,
                op0=mybir.AluOpType.add, op1=mybir.AluOpType.max,
            )
            nc.tensor.matmul(
                out=l_ps[:],
                lhsT=ht[:],
                rhs=w2_sb[:, hc * K:(hc + 1) * K],
                start=(hc == 0),
                stop=(hc == CH - 1),
            )
        out_sb = spool.tile([RPC, K], f16, tag="out")
        nc.vector.tensor_copy(out_sb[:], l_ps[:])
        nc.sync.dma_start(out_d[:], out_sb[:])

    nc.compile()
    return nc


def _get_nc(slots):
    if slots not in _CACHE:
        _CACHE[slots] = _build_nc(slots)
    return _CACHE[slots]


def _pick_slots(valid_mask, pos_span):
    """Slot count for this input: longest span that can touch real/zero rows."""
    span = np.asarray(pos_span).astype(np.int64)
    ln = int((span[:, 1] - span[:, 0]).max()) + 1
    return max(8, (ln + 3) // 4 * 4)


def _host_gather(enc16, valid_mask, pos_span, SLOTS):
    """Dense [B, SLOTS] token values per the slot semantics above -> fp16."""
    v = np.asarray(valid_mask).astype(np.int64) == 1          # [B, L]
    span = np.asarray(pos_span).astype(np.int64)              # [B, 2]
    s, e = span[:, 0], span[:, 1]
    nv = v.sum(axis=1)                                        # num valid per row
    order = np.argsort(~v, axis=1, kind="stable")             # valid tokens first
    q = s[:, None] + np.arange(SLOTS)[None, :]                # rank per slot
    qc = np.where(q <= e[:, None], q, s[:, None])             # padding -> slot 0
    use_zero = qc >= nv[:, None]                              # [B, SLOTS]
    toks = np.take_along_axis(order, np.minimum(qc, L - 1), axis=1)
    vals = enc16[np.arange(B)[:, None], toks]                 # [B, SLOTS, D]
    vals[use_zero] = np.float16(0.0)
    return vals


def _make_in_maps(inputs):
    enc16 = np.asarray(inputs["encoder_layers"], dtype=np.float32).astype(np.float16)
    W1 = np.asarray(inputs["W1"], dtype=np.float32)
    b1 = np.asarray(inputs["b1"], dtype=np.float32)
    W2 = np.asarray(inputs["W2"], dtype=np.float32)

    SLOTS = _pick_slots(inputs["valid_mask"], inputs["pos_span"])
    vals = _host_gather(enc16, inputs["valid_mask"], inputs["pos_span"], SLOTS)

    # device layouts: partition = d % 128, free = (chunk, ...)
    w1_dev = np.ascontiguousarray(
        W1.astype(np.float16).reshape(CH, 128, H).transpose(1, 0, 2).reshape(128, CH * H))
    w2_dev = np.ascontiguousarray(
        W2.astype(np.float16).reshape(CH, 128, K).transpose(1, 0, 2).reshape(128, CH * K))
    b1_dev = np.ascontiguousarray(b1.reshape(CH, 128).T)      # [128, CH] f32

    in_maps = []
    for c in range(NCORES):
        rows = slice(c * RPC, (c + 1) * RPC)
        # g[d%128, (c r j)] = vals[r, j, d]
        g = (vals[rows]                                       # [RPC, SLOTS, D]
             .transpose(2, 0, 1)                              # [D, RPC, SLOTS]
             .reshape(CH, 128, RPC * SLOTS)
             .transpose(1, 0, 2)
             .reshape(128, CH * RPC * SLOTS))
        in_maps.append({
            "g": np.ascontiguousarray(g),
            "b1": b1_dev, "w1": w1_dev, "w2": w2_dev,
        })
    return in_maps, SLOTS


def kernel(**inputs):
    from concourse.bass_utils import run_bass_kernel_spmd

    in_maps, slots = _make_in_maps(inputs)
    nc = _get_nc(slots)
    res = run_bass_kernel_spmd(nc, in_maps, list(range(NCORES)))
    out = np.concatenate(
        [res.results[c]["out"].astype(np.float32) for c in range(NCORES)], axis=0)

    b2 = np.asarray(inputs["b2"], dtype=np.float32)
    return (out + b2[None, :]).astype(np.float32)
